# revision 1
# baseline (speedup 1.0000x reference)
# Trainium2 Bass kernel for nn_Ml4fTransformer_48421461295652.
#
# Mathematical note (exact, architecture-level dead-code elimination):
# The decoder feature dim DD == 1, so every decoder LayerNorm normalizes over a
# single element: mean(x) == x exactly, so (x - mu) == 0 exactly, var == 0, and
# LN(x, g, b) == 0 * rsqrt(eps) * g + b == b, *exactly*, in any float precision
# and for ANY input values. In particular the final decoder LayerNorm output
# dec_out is dec_norm_b broadcast to (B, PRED). Hence the reference output is
#     out[b, j] = relu(dec_norm_b[0] * sum_k map_w[k, j] + map_b[j])
# for all b — independent of x, y, the whole encoder stack, the learn layer and
# every other weight. This identity holds for any inputs of these shapes, so
# computing it directly is an exact program transformation (verified vs the
# full reference: rel err ~2.5e-8, fp32 summation-order noise only).
#
# Sharding strategy: the live computation is a 64x64 column-sum + pointwise —
# a few microseconds of work. We replicate the three live tensors (map_w,
# map_b, dec_norm_b) to all 8 NeuronCores and run the identical tiny kernel
# SPMD on cores 0-7 (per-core compute, no collectives); core 0's output is the
# full (16, 64) result.
#
# On-device computation (per core), all in fp32:
#   ones[1,64]  = memset 1.0
#   pc[64,1]    = matmul(lhsT=ones[1,64], rhs=c[1,1])          # bcast c to 64 parts
#   cvec[64,1]  = copy(pc)
#   pr[1,64]    = matmul(lhsT=cvec[64,1], rhs=W[64,64], start)  # sum_k c*W[k,j]
#   pr[1,64]   += matmul(lhsT=ones[1,1],  rhs=b[1,64], stop)    # + map_b
#   row[1,64]   = max(pr, 0)                                    # ReLU
#   po[16,64]   = matmul(lhsT=ones[1,16], rhs=row[1,64])        # bcast to 16 rows
#   out         = copy(po) -> DMA to DRAM

import numpy as np

_B, _PRED = 16, 64
_N_CORES = 8

_cached = None  # (nc, input_names) — compile once per process


def _build_nc():
    import concourse.mybir as mybir
    import concourse.tile as tile
    from concourse import bacc

    fp32 = mybir.dt.float32
    nc = bacc.Bacc("TRN2", target_bir_lowering=False, debug=False)

    w_d = nc.dram_tensor("map_w", [64, 64], fp32, kind="ExternalInput")
    b_d = nc.dram_tensor("map_b", [1, 64], fp32, kind="ExternalInput")
    c_d = nc.dram_tensor("dec_norm_b", [1, 1], fp32, kind="ExternalInput")
    o_d = nc.dram_tensor("out", [_B, _PRED], fp32, kind="ExternalOutput")

    with tile.TileContext(nc) as tc:
        with (
            tc.tile_pool(name="sbuf", bufs=1) as pool,
            tc.tile_pool(name="psum", bufs=1, space="PSUM") as psum,
        ):
            W = pool.tile([64, 64], fp32)
            nc.sync.dma_start(W[:], w_d[:])
            Bb = pool.tile([1, 64], fp32)
            nc.sync.dma_start(Bb[:], b_d[:])
            C = pool.tile([1, 1], fp32)
            nc.sync.dma_start(C[:], c_d[:])

            ones = pool.tile([1, 64], fp32)
            nc.any.memset(ones[:], 1.0)

            # broadcast scalar c across 64 partitions via PE
            pc = psum.tile([64, 1], fp32)
            nc.tensor.matmul(pc[:], ones[:], C[:], start=True, stop=True)
            cvec = pool.tile([64, 1], fp32)
            nc.vector.tensor_copy(cvec[:], pc[:])

            # row_j = sum_k c * W[k, j]  (+ map_b via a second accumulating mm)
            pr = psum.tile([1, 64], fp32)
            nc.tensor.matmul(pr[:], cvec[:], W[:], start=True, stop=False)
            nc.tensor.matmul(pr[:], ones[:, :1], Bb[:], start=False, stop=True)

            # ReLU into SBUF
            row = pool.tile([1, 64], fp32)
            nc.vector.tensor_scalar_max(row[:], pr[:], 0.0)

            # broadcast the row to all 16 batch rows via PE
            po = psum.tile([_B, 64], fp32)
            nc.tensor.matmul(po[:], ones[:, :_B], row[:], start=True, stop=True)
            outt = pool.tile([_B, 64], fp32)
            nc.vector.tensor_copy(outt[:], po[:])
            nc.sync.dma_start(o_d[:], outt[:])

    nc.compile()
    return nc


def _get_nc():
    global _cached
    if _cached is None:
        _cached = _build_nc()
    return _cached


def _run(inputs, trace=False, **kw):
    from concourse.bass_utils import run_bass_kernel_spmd

    nc = _get_nc()
    in_map = {
        "map_w": np.ascontiguousarray(np.asarray(inputs["map_w"], dtype=np.float32)),
        "map_b": np.asarray(inputs["map_b"], dtype=np.float32).reshape(1, 64),
        "dec_norm_b": np.asarray(inputs["dec_norm_b"], dtype=np.float32).reshape(1, 1),
    }
    in_maps = [in_map for _ in range(_N_CORES)]
    res = run_bass_kernel_spmd(nc, in_maps, core_ids=list(range(_N_CORES)),
                               trace=trace, **kw)
    return res


def kernel(**inputs) -> np.ndarray:
    res = _run(inputs, trace=False)
    return np.asarray(res.results[0]["out"], dtype=np.float32)


# revision 2
# speedup vs baseline: 1.1156x; 1.1156x over previous
# Trainium2 Bass kernel for nn_Ml4fTransformer_48421461295652.
#
# Mathematical note (exact, architecture-level dead-code elimination):
# The decoder feature dim DD == 1, so every decoder LayerNorm normalizes over a
# single element: mean(x) == x exactly, so (x - mu) == 0 exactly, var == 0, and
# LN(x, g, b) == 0 * rsqrt(eps) * g + b == b, *exactly*, in any float precision
# and for ANY input values. In particular the final decoder LayerNorm output
# dec_out is dec_norm_b broadcast to (B, PRED) = (16, 64). Hence the reference
# output is
#     out[b, j] = relu(dec_norm_b[0] * sum_k map_w[k, j] + map_b[j])
# for all b — independent of x, y, the whole encoder stack, the learn layer and
# every other weight. This identity holds for any inputs of these shapes, so
# computing it directly is an exact program transformation (verified against
# the full fp32 reference: rel err ~2e-8, fp32 summation-order noise only).
#
# Sharding strategy: the live computation is a 64x64 column-sum + pointwise —
# microseconds of work. The three live tensors (map_w, map_b, dec_norm_b) are
# packed into one (65, 65) array, replicated to all 8 NeuronCores, and the
# identical tiny kernel runs SPMD on cores 0-7 (per-core compute, no
# collectives). Each core emits the unique [1, 64] row; the unshard step
# broadcasts it to the (16, 64) full output (all 16 batch rows are
# mathematically identical).
#
# On-device computation (per core), all fp32:
#   T[65,65]   <- DMA of packed input (rows 0..63 = map_w; row 64 = [c, map_b])
#   ones[64,1] = memset 1.0
#   S[1,64]    = matmul(lhsT=ones, rhs=T[0:64, 0:64])     # colsum_k map_w[k,j]
#   r[1,64]    = (S * T[64,0:1]) + T[64, 1:65]            # *c + map_b (one DVE op)
#   row[1,64]  = max(r, 0)                                # ReLU
#   DMA row -> DRAM

import numpy as np

_B, _PRED = 16, 64
_N_CORES = 8

_cached = None  # compiled Bass module — compile once per process


def _build_nc():
    import concourse.mybir as mybir
    import concourse.tile as tile
    from concourse import bacc

    fp32 = mybir.dt.float32
    nc = bacc.Bacc("TRN2", target_bir_lowering=False, debug=False)

    p_d = nc.dram_tensor("packed", [65, 65], fp32, kind="ExternalInput")
    o_d = nc.dram_tensor("out", [1, _PRED], fp32, kind="ExternalOutput")

    with tile.TileContext(nc) as tc:
        with (
            tc.tile_pool(name="sbuf", bufs=1) as pool,
            tc.tile_pool(name="psum", bufs=1, space="PSUM") as psum,
        ):
            T = pool.tile([65, 65], fp32)
            nc.sync.dma_start(T[:], p_d[:])

            ones = pool.tile([64, 1], fp32)
            nc.any.memset(ones[:], 1.0)

            S = psum.tile([1, 64], fp32)
            nc.tensor.matmul(S[:], ones[:], T[:64, :64], start=True, stop=True)

            row = pool.tile([1, 64], fp32)
            # row = (S * c) + map_b   (c = T[64, 0:1], map_b = T[64, 1:65])
            nc.vector.scalar_tensor_tensor(
                row[:], S[:], T[64:65, 0:1], T[64:65, 1:65],
                mybir.AluOpType.mult, mybir.AluOpType.add,
            )
            nc.vector.tensor_scalar_max(row[:], row[:], 0.0)

            nc.sync.dma_start(o_d[:], row[:])

    nc.compile()
    return nc


def _get_nc():
    global _cached
    if _cached is None:
        _cached = _build_nc()
    return _cached


def _pack(inputs):
    packed = np.empty((65, 65), dtype=np.float32)
    packed[:64, :64] = np.asarray(inputs["map_w"], dtype=np.float32)
    packed[:64, 64] = 0.0
    packed[64, 0] = np.asarray(inputs["dec_norm_b"], dtype=np.float32).reshape(())
    packed[64, 1:65] = np.asarray(inputs["map_b"], dtype=np.float32).reshape(64)
    return packed


def _run(inputs, trace=False, **kw):
    from concourse.bass_utils import run_bass_kernel_spmd

    nc = _get_nc()
    in_map = {"packed": _pack(inputs)}
    in_maps = [in_map for _ in range(_N_CORES)]
    return run_bass_kernel_spmd(nc, in_maps, core_ids=list(range(_N_CORES)),
                                trace=trace, **kw)


def _unshard(res):
    row = np.asarray(res.results[0]["out"], dtype=np.float32).reshape(1, _PRED)
    return np.ascontiguousarray(np.broadcast_to(row, (_B, _PRED)))


def kernel(**inputs) -> np.ndarray:
    return _unshard(_run(inputs, trace=False))


# revision 3
# speedup vs baseline: 1.1440x; 1.0255x over previous
# Trainium2 Bass kernel for nn_Ml4fTransformer_48421461295652.
#
# Mathematical note (exact, architecture-level dead-code elimination):
# The decoder feature dim DD == 1, so every decoder LayerNorm normalizes over a
# single element: mean(x) == x exactly, so (x - mu) == 0 exactly, var == 0, and
# LN(x, g, b) == 0 * rsqrt(eps) * g + b == b, *exactly*, in any float precision
# and for ANY input values. In particular the final decoder LayerNorm output
# dec_out is dec_norm_b broadcast to (B, PRED) = (16, 64). Hence the reference
# output is
#     out[b, j] = relu(dec_norm_b[0] * sum_k map_w[k, j] + map_b[j])
# for all b — independent of x, y, the whole encoder stack, the learn layer and
# every other weight. This identity holds for any inputs of these shapes, so
# computing it directly is an exact program transformation (verified against
# the full fp32 reference: rel err ~2e-8, fp32 summation-order noise only).
#
# Sharding strategy: the live computation is a 64x64 column-sum + pointwise —
# microseconds of work. The three live tensors (map_w, map_b, dec_norm_b) are
# packed into one (65, 65) array, replicated to all 8 NeuronCores, and the
# identical tiny kernel runs SPMD on cores 0-7 (per-core compute, no
# collectives). Each core emits the unique [1, 64] row; the unshard step
# broadcasts it to the (16, 64) full output (all 16 batch rows are
# mathematically identical).
#
# On-device computation (per core), all fp32:
#   T[65,65]   <- DMA of packed input (rows 0..63 = map_w; row 64 = [c, map_b])
#   ones[64,1] = memset 1.0
#   S[1,64]    = matmul(lhsT=ones, rhs=T[0:64, 0:64])     # colsum_k map_w[k,j]
#   r[1,64]    = (S * T[64,0:1]) + T[64, 1:65]            # *c + map_b (one DVE op)
#   row[1,64]  = max(r, 0)                                # ReLU
#   DMA row -> DRAM

import numpy as np

_B, _PRED = 16, 64
_N_CORES = 8

_cached = None  # compiled Bass module — compile once per process


def _build_nc():
    import concourse.mybir as mybir
    import concourse.tile as tile
    from concourse import bacc

    fp32 = mybir.dt.float32
    nc = bacc.Bacc("TRN2", target_bir_lowering=False, debug=False)

    p_d = nc.dram_tensor("packed", [65, 65], fp32, kind="ExternalInput")
    o_d = nc.dram_tensor("out", [1, _PRED], fp32, kind="ExternalOutput")

    with tile.TileContext(nc) as tc:
        with (
            tc.tile_pool(name="sbuf", bufs=1) as pool,
            tc.tile_pool(name="psum", bufs=1, space="PSUM") as psum,
        ):
            T = pool.tile([65, 65], fp32)
            nc.sync.dma_start(T[:], p_d[:])

            ones = pool.tile([64, 1], fp32)
            nc.any.memset(ones[:], 1.0)

            S = psum.tile([1, 64], fp32)
            nc.tensor.matmul(S[:], ones[:], T[:64, :64], start=True, stop=True)

            row = pool.tile([1, 64], fp32)
            # row = (S * c) + map_b   (c = T[64, 0:1], map_b = T[64, 1:65])
            nc.vector.scalar_tensor_tensor(
                row[:], S[:], T[64:65, 0:1], T[64:65, 1:65],
                mybir.AluOpType.mult, mybir.AluOpType.add,
            )
            nc.vector.tensor_scalar_max(row[:], row[:], 0.0)

            nc.sync.dma_start(o_d[:], row[:])

    nc.compile()
    return nc


def _get_nc():
    global _cached
    if _cached is None:
        _cached = _build_nc()
    return _cached


def _pack(inputs):
    packed = np.empty((65, 65), dtype=np.float32)
    packed[:64, :64] = np.asarray(inputs["map_w"], dtype=np.float32)
    packed[:64, 64] = 0.0
    packed[64, 0] = np.asarray(inputs["dec_norm_b"], dtype=np.float32).reshape(())
    packed[64, 1:65] = np.asarray(inputs["map_b"], dtype=np.float32).reshape(64)
    return packed


def _run(inputs, trace=False, **kw):
    from concourse.bass_utils import run_bass_kernel_spmd

    nc = _get_nc()
    in_map = {"packed": _pack(inputs)}
    in_maps = [in_map for _ in range(_N_CORES)]
    try:
        return run_bass_kernel_spmd(nc, in_maps, core_ids=list(range(_N_CORES)),
                                    trace=trace, **kw)
    except Exception:
        # one retry — transient device-state failures (e.g. a previous process
        # crashed mid-execution and left a core wedged) clear on re-run
        return run_bass_kernel_spmd(nc, in_maps, core_ids=list(range(_N_CORES)),
                                    trace=trace, **kw)


def _unshard(res):
    row = np.asarray(res.results[0]["out"], dtype=np.float32).reshape(1, _PRED)
    return np.ascontiguousarray(np.broadcast_to(row, (_B, _PRED)))


def kernel(**inputs) -> np.ndarray:
    return _unshard(_run(inputs, trace=False))
